# revision 21
# baseline (speedup 1.0000x reference)
"""Trainium2 Bass kernel for nn_DecoderGenerator (2-layer LSTM decoder +
attention (buggy softmax-over-batch) + vocab FC + CE loss over T=63 steps).

Sharding (8 NeuronCores, SPMD, single launch):
  - LSTM recurrence replicated on all cores (bf16 matmuls, fp32 gate math).
  - Attention sharded over encoder positions (8 per core, zero-padded).
  - Partial attention context vectors summed with one AllReduce (bf16).
  - FC to vocab sharded over vocab (4000/core); CE sum-exp partials returned
    per core; host combines partials + target-logit dots into the scalar.
"""
import os
import sys
import types

import numpy as np
import ml_dtypes

import concourse.mybir as mybir
import concourse.tile as tile
from concourse import bacc
from concourse.bass_utils import run_bass_kernel_spmd

BF16 = mybir.dt.bfloat16
FP8 = mybir.dt.float8e4
F32 = mybir.dt.float32
AF = mybir.ActivationFunctionType

NCORES = 8
B = 64
V = 32000
VS = V // NCORES     # 4000
ES = 8               # encoder positions per core (zero-padded)
NCH = 8              # vocab N-chunks per shard
CH = VS // NCH       # 500

# h-feature order induced by the two 128-col PE transposes of [_, 256] state
PERM = np.r_[0:128, 256:384, 128:256, 384:512]

_CACHE = {}
last_exec_time_ns = None


def _maybe_install_trace_shim():
    try:
        import antenv
        if "antenv.axon_hooks" not in sys.modules:
            mod = types.ModuleType("antenv.axon_hooks")
            holder = [None]
            mod.set_axon_ntff_profile_hook = lambda h: holder.__setitem__(0, h)
            mod.get_axon_ntff_profile_hook = lambda: holder[0]
            sys.modules["antenv.axon_hooks"] = mod
            antenv.axon_hooks = mod
            from trn_agent_boot.trn_boot import _ntff_profile_via_ctypes
            mod.set_axon_ntff_profile_hook(
                _ntff_profile_via_ctypes("/opt/axon/libaxon_pjrt.so"))
        return True
    except Exception:
        return False


def _bf(x):
    return np.ascontiguousarray(
        np.asarray(x, np.float32).astype(ml_dtypes.bfloat16))


def _gate_cols(q):
    # free-dim order per half q: [g, i, f, o] blocks of 256
    return np.r_[1024 + q * 256:1024 + q * 256 + 256,
                 0 + q * 256:0 + q * 256 + 256,
                 512 + q * 256:512 + q * 256 + 256,
                 1536 + q * 256:1536 + q * 256 + 256]


def _weight_rhs(WxT, WhT):
    """WxT/WhT: [512, 2048] pre-transposed (and row-permuted as needed).
    -> [8, 128, 2, 1024] (ktile, kpart, half, gatecols)."""
    out = np.empty((8, 128, 2, 1024), np.float32)
    for q in range(2):
        cols = _gate_cols(q)
        for kt in range(4):
            out[kt, :, q, :] = WxT[kt * 128:(kt + 1) * 128][:, cols]
            out[kt + 4, :, q, :] = WhT[kt * 128:(kt + 1) * 128][:, cols]
    return out


def _lstm_cell(nc, gp, g, c_cur, c_new):
    """Gate math. g: PSUM [128, 1024] = [g|i|f|o]x256. Returns h (bf16)."""
    tg = gp.tile([128, 256], F32, tag="tg")
    nc.scalar.activation(out=tg, in_=g[:, 0:256], func=AF.Tanh)
    sio = gp.tile([128, 768], F32, tag="sio")
    nc.scalar.activation(out=sio, in_=g[:, 256:1024], func=AF.Sigmoid)
    ig = gp.tile([128, 256], F32, tag="ig")
    nc.vector.tensor_mul(ig, sio[:, 0:256], tg)
    fc = gp.tile([128, 256], F32, tag="fc")
    nc.vector.tensor_mul(fc, sio[:, 256:512], c_cur)
    nc.vector.tensor_add(c_new, ig, fc)
    tc_ = gp.tile([128, 256], F32, tag="tc_")
    nc.scalar.activation(out=tc_, in_=c_new, func=AF.Tanh)
    h = gp.tile([128, 256], BF16, tag="h")
    nc.vector.tensor_mul(h, sio[:, 512:768], tc_)
    return h


def _gate_matmuls(nc, g, on1, bb, w, lhs_lo, lhs_hi):
    """g = bias + W @ [x; h]. Col-tiled halves interleaved kt-outer so the
    two 64-wide column groups stream concurrently; bias folded into the PSUM
    accumulation as a K=1 matmul so the ACT reads PSUM directly."""
    for n in range(2):
        nc.tensor.matmul(
            g[:, 512 * n:512 * n + 512], on1,
            bb[:, 512 * n:512 * n + 512],
            start=True, stop=False, tile_position=(0, 0),
            skip_group_check=True)
        for kt in range(8):
            lhs = lhs_lo(kt) if kt < 4 else lhs_hi(kt - 4)
            for q in range(2):
                nc.tensor.matmul(
                    g[64 * q:64 * q + 64, 512 * n:512 * n + 512], lhs,
                    w[:, kt, q, 512 * n:512 * n + 512],
                    start=False, stop=(kt == 7),
                    tile_position=(0, 64 * q))


def build_program(T):
    nc = bacc.Bacc(None, target_bir_lowering=False, debug=False,
                   num_devices=NCORES)
    R = B * T
    MT = 2 * T            # rows per FC M-tile (2 batches' worth)
    NMT = R // MT         # 32

    ei = lambda n, s, d=BF16: nc.dram_tensor(n, s, d, kind="ExternalInput")
    eT_all = ei("eT_all", [T, 128, 4, B])
    w1t = ei("w1t", [128, 8, 2, 1024])
    w2t = ei("w2t", [128, 8, 2, 1024])
    wqt = ei("wqt", [128, 4, 2, 256])
    bias1 = ei("bias1", [2, 1024])
    bias2 = ei("bias2", [2, 1024])
    ones1 = ei("ones1", [2, 128])
    id128 = ei("id128", [128, 128])
    encT = ei("encT", [128, 4, ES * B])
    weT = ei("weT", [128, 4, 4, 128])
    attnbT = ei("attnbT", [128, 4], F32)
    vwT = ei("vwT", [128, 4, 1])
    encE = ei("encE", [B, ES, 512])
    fcw = nc.dram_tensor("fcw", [128, 8, VS], FP8, kind="ExternalInput")

    out_semp = nc.dram_tensor("out_semp", [MT, NMT], F32, kind="ExternalOutput")
    out_top = nc.dram_tensor("out_top", [128, 4 * B * T], BF16,
                             kind="ExternalOutput")
    out_wtd = nc.dram_tensor("out_wtd", [128, 4 * B * T], BF16,
                             kind="ExternalOutput")

    scoresE = nc.dram_tensor("scoresE", [T, ES * B], F32)
    attD = nc.dram_tensor("attD", [T, ES * B], BF16)
    war_in = [nc.dram_tensor(f"war_in{c}", [128, 4, 16, T], BF16)
              for c in range(4)]
    war_out = [nc.dram_tensor(f"war_out{c}", [128, 4, 16, T], BF16,
                              addr_space="Shared") for c in range(4)]

    with tile.TileContext(nc) as tc:
        with tc.tile_pool(name="persist", bufs=1) as pp:
            topT = pp.tile([128, 4, B, T], BF16, tag="topT")
            wtd = pp.tile([128, 4, B, T], BF16, tag="wtd")

            # ---------------- phase 1: recurrence ----------------
            with (
                tc.tile_pool(name="pw", bufs=1) as pw,
                tc.tile_pool(name="roll", bufs=3) as rp,
                tc.tile_pool(name="gp", bufs=2) as gp,
                tc.tile_pool(name="psA", bufs=2, space="PSUM") as psA,
                tc.tile_pool(name="psB", bufs=1, space="PSUM") as psB,
                tc.tile_pool(name="psT", bufs=1, space="PSUM") as psT,
                tc.tile_pool(name="psQ", bufs=1, space="PSUM") as psQ,
            ):
                w1 = pw.tile([128, 8, 2, 1024], BF16, tag="w1")
                nc.sync.dma_start(out=w1,
                                  in_=w1t.ap())
                w2 = pw.tile([128, 8, 2, 1024], BF16, tag="w2")
                nc.sync.dma_start(out=w2,
                                  in_=w2t.ap())
                wq = pw.tile([128, 4, 2, 256], BF16, tag="wq")
                nc.sync.dma_start(out=wq,
                                  in_=wqt.ap())
                b1 = pw.tile([2, 1024], BF16, tag="b1")
                nc.sync.dma_start(out=b1, in_=bias1.ap())
                b2 = pw.tile([2, 1024], BF16, tag="b2")
                nc.sync.dma_start(out=b2, in_=bias2.ap())
                on1 = pw.tile([2, 128], BF16, tag="on1")
                nc.sync.dma_start(out=on1, in_=ones1.ap())
                idm = pw.tile([128, 128], BF16, tag="idm")
                nc.sync.dma_start(out=idm, in_=id128.ap())
                abT = pw.tile([128, 4], F32, tag="abT")
                nc.sync.dma_start(out=abT, in_=attnbT.ap())
                vw = pw.tile([128, 4, 1], BF16, tag="vw")
                nc.sync.dma_start(out=vw,
                                  in_=vwT.ap())
                epj = pw.tile([128, 4, ES, B], BF16, tag="epj")

                h1T = [pw.tile([128, 256], BF16, tag=f"h1T{i}", name=f"h1T{i}")
                       for i in (0, 1)]
                h2T = [pw.tile([128, 256], BF16, tag=f"h2T{i}", name=f"h2T{i}")
                       for i in (0, 1)]
                c1 = [pw.tile([128, 256], F32, tag=f"c1{i}", name=f"c1{i}")
                      for i in (0, 1)]
                c2 = [pw.tile([128, 256], F32, tag=f"c2{i}", name=f"c2{i}")
                      for i in (0, 1)]
                for s in (*h1T, *h2T, *c1, *c2):
                    nc.vector.memset(s, 0.0)

                # enc_proj = We @ encT (+ attn_b)
                wes = pw.tile([128, 4, 4, 128], BF16, tag="wes")
                nc.sync.dma_start(out=wes,
                                  in_=weT.ap())
                ets = pw.tile([128, 4, ES * B], BF16, tag="ets")
                nc.sync.dma_start(out=ets, in_=encT.ap())
                for ht in range(4):
                    pj = psQ.tile([128, ES * B], F32, tag="psq")
                    for kt in range(4):
                        nc.tensor.matmul(pj, wes[:, kt, ht, :], ets[:, kt],
                                         start=(kt == 0), stop=(kt == 3))
                    nc.scalar.activation(
                        out=epj[:, ht].rearrange("p e b -> p (e b)"), in_=pj,
                        func=AF.Identity, bias=abT[:, ht:ht + 1], scale=1.0)

                def attn_q(hT):
                    # q = Wh @ top ; returns qT [128, 4, B] bf16
                    qp = psQ.tile([128, 256], F32, tag="psq", name="qp")
                    for kt in range(4):
                        for q in range(2):
                            nc.tensor.matmul(
                                qp[64 * q:64 * q + 64, :],
                                hT[:, 64 * kt:64 * kt + 64],
                                wq[:, kt, q], start=(kt == 0), stop=(kt == 3),
                                tile_position=(0, 64 * q))
                    qsb = rp.tile([128, 256], BF16, tag="qsb", name="qsb")
                    nc.vector.tensor_copy(qsb, qp)
                    qps = psQ.tile([128, 256], BF16, tag="psq", name="qps")
                    nc.tensor.transpose(qps[:, 0:128], qsb[:, 0:128], idm)
                    nc.tensor.transpose(qps[:, 128:256], qsb[:, 128:256], idm)
                    qT = rp.tile([128, 4, B], BF16, tag="qT", name="qT")
                    nc.vector.tensor_copy(
                        qT, qps.rearrange("p (k b) -> p k b", k=4))
                    return qT

                def attn_scores(qT, t):
                    en = rp.tile([128, ES, 4, B], BF16, tag="en", name="en")
                    for e in range(ES):
                        nc.vector.tensor_add(en[:, e], epj[:, :, e, :], qT)
                    enf = en.rearrange("p e k b -> p (e k b)")
                    nc.scalar.activation(out=enf, in_=enf, func=AF.Tanh)
                    scr = psQ.tile([1, ES * B], F32, tag="psq", name="scr")
                    for kt in range(4):
                        nc.tensor.matmul(
                            scr, vw[:, kt], en[:, :, kt, :],
                            start=(kt == 0), stop=(kt == 3))
                    ssb = rp.tile([1, ES * B], F32, tag="ssb", name="ssb")
                    nc.vector.tensor_copy(ssb, scr)
                    nc.sync.dma_start(out=scoresE.ap()[t:t + 1, :], in_=ssb)

                qT_prev = None
                for t in range(T):
                    cur, nxt = t % 2, (t + 1) % 2
                    et = rp.tile([128, 4, B], BF16, tag="et")
                    nc.sync.dma_start(
                        out=et, in_=eT_all.ap()[t])

                    g1 = psA.tile([128, 1024], F32, tag="g1")
                    _gate_matmuls(
                        nc, g1, on1, b1, w1,
                        lambda kt: et[:, kt],
                        lambda j: h1T[cur][:, 64 * j:64 * j + 64])
                    # attention q-projection of the previous step fills the
                    # PE stall while the layer-1 gate chain runs
                    if t > 0:
                        qT_prev = attn_q(h2T[cur])
                    h1n = _lstm_cell(nc, gp, g1, c1[cur], c1[nxt])
                    tps = psT.tile([128, 256], BF16, tag="pst")
                    nc.tensor.transpose(tps[:, 0:128], h1n[:, 0:128], idm)
                    nc.tensor.transpose(tps[:, 128:256], h1n[:, 128:256], idm)
                    nc.vector.tensor_copy(h1T[nxt], tps)

                    g2 = psB.tile([128, 1024], F32, tag="g2")
                    _gate_matmuls(
                        nc, g2, on1, b2, w2,
                        lambda kt: h2T[cur][:, 64 * kt:64 * kt + 64],
                        lambda j: h1T[nxt][:, 64 * j:64 * j + 64])
                    # previous step's energy/tanh/scores fill the l2 stall
                    if t > 0:
                        attn_scores(qT_prev, t - 1)
                    h2n = _lstm_cell(nc, gp, g2, c2[cur], c2[nxt])
                    tps2 = psT.tile([128, 256], BF16, tag="pst")
                    nc.tensor.transpose(tps2[:, 0:128], h2n[:, 0:128], idm)
                    nc.tensor.transpose(tps2[:, 128:256], h2n[:, 128:256], idm)
                    nc.vector.tensor_copy(h2T[nxt], tps2)
                    nc.scalar.activation(
                        out=topT[:, :, :, t],
                        in_=tps2.rearrange("p (k b) -> p k b", k=4),
                        func=AF.Identity, scale=1.0)

                # flush final step's attention
                attn_scores(attn_q(h2T[T % 2]), T - 1)

            # ---------------- tail ----------------
            with (
                tc.tile_pool(name="tail", bufs=2) as tp,
                tc.tile_pool(name="fcs", bufs=3) as fs,
                tc.tile_pool(name="psF", bufs=4, space="PSUM") as psF,
                tc.tile_pool(name="psW", bufs=2, space="PSUM") as psW,
            ):
                # softmax over batch per (t, e)
                sc2 = tp.tile([T, ES, B], F32, tag="sc2")
                nc.sync.dma_start(
                    out=sc2.rearrange("t e b -> t (e b)"), in_=scoresE.ap())
                ex = tp.tile([T, ES, B], F32, tag="ex")
                nc.scalar.activation(out=ex.rearrange("t e b -> t (e b)"),
                                     in_=sc2.rearrange("t e b -> t (e b)"),
                                     func=AF.Exp)
                dsum = tp.tile([T, ES], F32, tag="dsum")
                nc.vector.reduce_sum(out=dsum, in_=ex,
                                     axis=mybir.AxisListType.X)
                rd = tp.tile([T, ES], F32, tag="rd")
                nc.vector.reciprocal(out=rd, in_=dsum)
                nld = tp.tile([T, ES], F32, tag="nld")
                nc.scalar.activation(out=nld, in_=rd, func=AF.Ln)
                attS = tp.tile([T, ES, B], BF16, tag="attS")
                for e in range(ES):
                    nc.scalar.activation(out=attS[:, e], in_=sc2[:, e],
                                         func=AF.Exp, bias=nld[:, e:e + 1],
                                         scale=1.0)
                nc.sync.dma_start(out=attD.ap(),
                                  in_=attS.rearrange("t e b -> t (e b)"))
                attR = tp.tile([8, T, B], BF16, tag="attR")
                nc.sync.dma_start(
                    out=attR, in_=attD.ap().rearrange("t (e b) -> e t b", e=ES))

                # z top half can cast immediately (unblocks FC kp 0-1)
                z8 = fs.tile([128, 8, B * T], FP8, tag="z8")
                nc.vector.tensor_copy(
                    z8[:, 0:4], topT.rearrange("p k b t -> p k (b t)"))
                nc.sync.dma_start(out=out_top.ap(),
                                  in_=topT.rearrange("p k b t -> p (k b t)"))

                # weighted_partial[h, b, t] = sum_e att * enc, with the
                # AllReduce chunked over batch groups of 16 so FC M-tiles
                # start as soon as their batch pair's context arrives
                for c in range(4):
                    for b in range(16 * c, 16 * c + 16):
                        eb_ = fs.tile([8, 4, 128], BF16, tag="encb")
                        nc.sync.dma_start(
                            out=eb_,
                            in_=encE.ap()[b].rearrange("e (h p) -> e h p", h=4))
                        wps = psW.tile([128, 4, T], F32, tag="wps")
                        for ht in range(4):
                            nc.tensor.matmul(wps[:, ht], eb_[:, ht, :],
                                             attR[:, :, b], start=True,
                                             stop=True)
                        nc.vector.tensor_copy(wtd[:, :, b, :], wps)
                    bs = slice(16 * c, 16 * c + 16)
                    nc.sync.dma_start(out=war_in[c].ap(),
                                      in_=wtd[:, :, bs, :])
                    nc.gpsimd.collective_compute(
                        "AllReduce", mybir.AluOpType.add,
                        replica_groups=[list(range(NCORES))],
                        ins=[war_in[c].ap().opt()],
                        outs=[war_out[c].ap().opt()])
                    nc.sync.dma_start(out=wtd[:, :, bs, :],
                                      in_=war_out[c].ap())
                    nc.vector.tensor_copy(
                        z8[:, 4:8, 16 * c * T:(16 * c + 16) * T].rearrange(
                            "p k (b t) -> p k b t", b=16),
                        wtd[:, :, bs, :])

                nc.sync.dma_start(out=out_wtd.ap(),
                                  in_=wtd.rearrange("p k b t -> p (k b t)"))
                sump = fs.tile([MT, NMT * NCH], F32, tag="sump")
                for nk in range(NCH):
                    fw = fs.tile([128, 8, CH], FP8, tag="fw")
                    nc.sync.dma_start(
                        out=fw,
                        in_=fcw.ap()[:, :, nk * CH:(nk + 1) * CH])
                    for m in range(NMT):
                        pf = psF.tile([MT, CH], F32, tag="pf")
                        for kp in range(4):
                            nc.tensor.matmul(
                                pf,
                                z8[:, 2 * kp:2 * kp + 2,
                                   m * MT:(m + 1) * MT],
                                fw[:, 2 * kp:2 * kp + 2],
                                start=(kp == 0), stop=(kp == 3),
                                perf_mode=mybir.MatmulPerfMode.DoubleRow)
                        ebx = fs.tile([MT, CH], BF16, tag="ebx")
                        nc.scalar.activation(
                            out=ebx, in_=pf, func=AF.Exp,
                            accum_out=sump[:, m * NCH + nk:m * NCH + nk + 1])
                semp = fs.tile([MT, NMT], F32, tag="semp")
                nc.vector.reduce_sum(
                    out=semp, in_=sump.rearrange("p (m n) -> p m n", m=NMT),
                    axis=mybir.AxisListType.X)
                nc.sync.dma_start(out=out_semp.ap(), in_=semp)
    nc.finalize()
    return nc


def _prep_inputs(X, enc, emb, Wih, Whh, bih, bhh, aWh, aWe, ab, vw, fcW):
    Bn, S = X.shape
    T = S - 1
    E = np.asarray(emb, np.float32)[np.asarray(X[:, :T], np.int64)]  # [B,T,D]
    eT = _bf(E.transpose(1, 2, 0).reshape(T, 4, 128, Bn).transpose(0, 2, 1, 3))

    w1 = _bf(_weight_rhs(Wih[0].T, Whh[0].T[PERM, :]).transpose(1, 0, 2, 3))
    w2 = _bf(_weight_rhs(Whh[1].T[PERM, :], Wih[1].T[PERM, :]).transpose(1, 0, 2, 3))
    b1 = np.stack([(bih[0] + bhh[0])[_gate_cols(q)] for q in range(2)])
    b2 = np.stack([(bih[1] + bhh[1])[_gate_cols(q)] for q in range(2)])
    wqt = np.empty((4, 128, 2, 256), np.float32)
    WhT = aWh.T[PERM, :]
    for kt in range(4):
        for qh in range(2):
            wqt[kt, :, qh, :] = WhT[kt * 128:(kt + 1) * 128,
                                    qh * 256:(qh + 1) * 256]
    weT = np.empty((4, 128, 4, 128), np.float32)
    WeT = aWe.T
    for kt in range(4):
        for ht in range(4):
            weT[kt, :, ht, :] = WeT[kt * 128:(kt + 1) * 128,
                                    PERM[ht * 128:(ht + 1) * 128]]
    abT = np.empty((128, 4), np.float32)
    for ht in range(4):
        abT[:, ht] = ab[PERM[ht * 128:(ht + 1) * 128]]
    vwT = vw[PERM].reshape(4, 128, 1)
    fcT = fcW.T[np.r_[PERM, 512:1024], :]  # [1024, V], rows in z order

    common = dict(
        eT_all=eT, w1t=w1, w2t=w2, wqt=_bf(wqt.transpose(1, 0, 2, 3)),
        bias1=_bf(b1), bias2=_bf(b2), id128=_bf(np.eye(128)),
        ones1=_bf(np.vstack([np.r_[np.ones(64), np.zeros(64)],
                             np.r_[np.zeros(64), np.ones(64)]])),
        weT=_bf(weT.transpose(1, 0, 2, 3)),
        attnbT=np.ascontiguousarray(abT),
        vwT=_bf(vwT.transpose(1, 0, 2)),
    )
    in_maps = []
    for c in range(NCORES):
        enc_pad = np.zeros((Bn, ES, 512), np.float32)
        e0 = c * ES
        n = min(ES, T - e0)
        if n > 0:
            enc_pad[:, :n, :] = enc[:, e0:e0 + n, :]
        encTc = _bf(enc_pad.transpose(2, 1, 0).reshape(4, 128, ES * Bn).transpose(1, 0, 2))
        encEc = _bf(enc_pad)
        fcs = np.ascontiguousarray(
            fcT[:, c * VS:(c + 1) * VS].reshape(8, 128, VS).transpose(1, 0, 2)
            .astype(ml_dtypes.float8_e4m3))
        in_maps.append(dict(common, encT=encTc, encE=encEc, fcw=fcs))
    return in_maps, T


def kernel(X, encoderOutputs, mask, emb, lstm_Wih, lstm_Whh, lstm_bih,
           lstm_bhh, attn_Wh, attn_We, attn_b, v_w, fc_W, fc_b):
    global last_exec_time_ns
    X = np.asarray(X)
    mask = np.asarray(mask)
    assert not mask.any(), "nonzero mask not supported by this kernel"
    fc_b = np.asarray(fc_b, np.float32)
    assert not fc_b.any(), "nonzero fc_b not supported by this kernel"
    enc = np.asarray(encoderOutputs, np.float32)
    Bn, S = X.shape
    T = S - 1

    in_maps, T = _prep_inputs(
        X, enc, emb, np.asarray(lstm_Wih, np.float32),
        np.asarray(lstm_Whh, np.float32), np.asarray(lstm_bih, np.float32),
        np.asarray(lstm_bhh, np.float32), np.asarray(attn_Wh, np.float32),
        np.asarray(attn_We, np.float32), np.asarray(attn_b, np.float32),
        np.asarray(v_w, np.float32), np.asarray(fc_W, np.float32))

    if T not in _CACHE:
        _CACHE[T] = build_program(T)
    nc = _CACHE[T]

    trace = bool(os.environ.get("KERNEL_TRACE"))
    if trace:
        trace = _maybe_install_trace_shim()
    res = run_bass_kernel_spmd(nc, in_maps, core_ids=list(range(NCORES)),
                               trace=trace)
    last_exec_time_ns = res.exec_time_ns

    # ---- host combine ----
    MT = 2 * T
    sumexp = np.zeros((MT, Bn * T // MT), np.float64)
    for c in range(NCORES):
        sumexp += np.asarray(res.results[c]["out_semp"], np.float64)
    sumexp = sumexp.T.reshape(Bn * T)  # rows r = b*T + t

    r0 = res.results[0]
    top = np.asarray(r0["out_top"], np.float32).reshape(128, 4, Bn, T)
    wtd = np.asarray(r0["out_wtd"], np.float32).reshape(128, 4, Bn, T)
    # z in chunk order: features [PERM(top) | natural(weighted)]
    z = np.concatenate([top.transpose(2, 3, 1, 0).reshape(Bn, T, 512),
                        wtd.transpose(2, 3, 1, 0).reshape(Bn, T, 512)], -1)

    tgt = np.asarray(X[:, 1:], np.int64)
    fcW_bf = np.asarray(fc_W, np.float32).astype(
        ml_dtypes.bfloat16).astype(np.float32)
    Wt = fcW_bf[tgt][:, :, np.r_[PERM, 512:1024]]
    dot = (z.astype(np.float64) * Wt).sum(-1) + fc_b[tgt]

    nll = np.log(sumexp.reshape(Bn, T)) - dot
    valid = tgt != 0
    loss_t = (nll * valid).sum(0) / valid.sum(0)
    return np.float32(loss_t.mean())
